# revision 11
# baseline (speedup 1.0000x reference)
"""Trainium2 Bass kernel for nn_NeuralODE: GRU encoder + reparameterized
sample + 4x adaptive Dopri5 ODE intervals + linear head. Pure data-parallel
over 8 NeuronCores (batch 1024 -> 128/core).

Self-contained: hardcodes shapes/sharding; only needs concourse + numpy.

Layout (per core): feature-major. A [F, Bc] tensor with F=256 lives in SBUF as
[128, 2*128] where tile[p, c*128+b] = X[c*128+p, b]. Matmul stationaries are
W.T chunk-interleaved: wT[p, ((ic*njc)+jc)*128+j] = W.T[ic*128+p, jc*128+j].

The adaptive Dopri5 runs S_FAST steps per interval (measured: 5 needed for the
reference data distribution) and writes per-interval "still active" flags; if
any flag fires, we rerun with the exact 32-step schedule (bitwise-faithful to
the reference masking semantics, so 32-step output == reference output).
"""
import math

import numpy as np

B, N_T, D_IN, M_META, H = 1024, 64, 4, 4, 256
NCORES = 8
BC = B // NCORES           # 128 samples per core
NI = 4                     # intervals
MAX_STEPS = 32
S_FAST = 6
ODE_ON = True  # debug: False skips ODE intervals
ATOL, RTOL = 1e-6, 1e-3

A21 = 0.2
A31, A32 = 3 / 40, 9 / 40
A41, A42, A43 = 44 / 45, -56 / 15, 32 / 9
A51, A52, A53, A54 = 19372 / 6561, -25360 / 2187, 64448 / 6561, -212 / 729
A61, A62, A63, A64, A65 = 9017 / 3168, -355 / 33, 46732 / 5247, 49 / 176, -5103 / 18656
BCO = (35 / 384, 0.0, 500 / 1113, 125 / 192, -2187 / 6784, 11 / 84, 0.0)
B4 = (5179 / 57600, 0.0, 7571 / 16695, 393 / 640, -92097 /339200, 187 / 2100, 1 / 40)
EC = tuple(b - b4 for b, b4 in zip(BCO, B4))

SELU_L = 1.0507009873554805
SELU_A = 1.6732632423543772
LN_ALPHA = float(np.log(SELU_A))

_BUILD_CACHE = {}
LAST_RESULTS = []  # BassKernelResults stash for test harnesses


def _chunked(x):
    """[F, B] -> [128, (F//128)*B] feature-chunk layout."""
    f, b = x.shape
    nch = f // 128
    return np.ascontiguousarray(
        x.reshape(nch, 128, b).transpose(1, 0, 2).reshape(128, nch * b))


def _lhsT(w):
    """W [O, I] -> [128, nic*njc*128] chunk-interleaved W.T for matmul lhsT."""
    o, i = w.shape
    nic, njc = i // 128, o // 128
    wt = w.T.reshape(nic, 128, njc, 128).transpose(1, 0, 2, 3)
    return np.ascontiguousarray(wt.reshape(128, nic * njc * 128))


def _build(S):
    """Build the Bass program for S dopri steps per interval."""
    import concourse.bacc as bacc
    import concourse.mybir as mybir
    from concourse.tile import TileContext

    F32 = mybir.dt.float32
    U8 = mybir.dt.uint8
    AF = mybir.ActivationFunctionType
    ALU = mybir.AluOpType

    nc = bacc.Bacc("TRN2", target_bir_lowering=False)

    # ---- DRAM I/O ----
    xmT = nc.dram_tensor("xmT", [D_IN + M_META, N_T * BC], F32, kind="ExternalInput")
    epsT = nc.dram_tensor("epsT", [128, 2 * BC], F32, kind="ExternalInput")
    t0r_d = nc.dram_tensor("t0r", [NI, BC], F32, kind="ExternalInput")
    t1r_d = nc.dram_tensor("t1r", [NI, BC], F32, kind="ExternalInput")
    dsr_d = nc.dram_tensor("dsr", [NI, BC], F32, kind="ExternalInput")
    wihT_d = nc.dram_tensor("wihT", [D_IN + M_META, 3 * H], F32, kind="ExternalInput")
    whhT_d = nc.dram_tensor("whhT", [128, 2 * 3 * H], F32, kind="ExternalInput")
    encW1T_d = nc.dram_tensor("encW1T", [128, 2 * 2 * 128], F32, kind="ExternalInput")
    encW2T_d = nc.dram_tensor("encW2T", [128, 2 * 4 * 128], F32, kind="ExternalInput")
    odeWT_d = [nc.dram_tensor(f"odeW{l}T", [128, 2 * 2 * 128], F32, kind="ExternalInput")
               for l in range(1, 5)]
    onesc_d = nc.dram_tensor("onesc", [128, 1], F32, kind="ExternalInput")
    onesr_d = nc.dram_tensor("onesr", [1, BC], F32, kind="ExternalInput")
    yout_d = nc.dram_tensor("yout", [128, 2 * BC], F32, kind="ExternalOutput")
    flags_d = nc.dram_tensor("flags", [1, NI], F32, kind="ExternalOutput")

    GW = 3 * H  # 768 gate width

    with TileContext(nc) as tc:
        with tc.tile_pool(name="persist", bufs=1) as pp, \
             tc.tile_pool(name="work", bufs=1) as wp, \
             tc.tile_pool(name="psA", bufs=4, space="PSUM") as psA, \
             tc.tile_pool(name="psB", bufs=2, space="PSUM") as psB, \
             tc.tile_pool(name="psQ", bufs=1, space="PSUM") as psQ:

            # ---- load constants / weights / inputs ----
            def ld(shape, dram, tag):
                t = pp.tile(shape, F32, tag=tag, name=tag)
                nc.sync.dma_start(out=t[:], in_=dram[:, :])
                return t

            xm_sb = ld([D_IN + M_META, N_T * BC], xmT, "xm")
            eps_sb = ld([128, 2 * BC], epsT, "eps")
            wih_sb = ld([D_IN + M_META, GW], wihT_d, "wih")
            whh_sb = ld([128, 2 * GW], whhT_d, "whh")
            e1_sb = ld([128, 2 * 2 * 128], encW1T_d, "encw1")
            e2_sb = ld([128, 2 * 4 * 128], encW2T_d, "encw2")
            ow_sb = [ld([128, 2 * 2 * 128], odeWT_d[l], f"odew{l}") for l in range(4)]
            onesc = ld([128, 1], onesc_d, "onesc")
            onesr = ld([1, BC], onesr_d, "onesr")

            flags_sb = pp.tile([1, NI], F32, tag="flags")
            nc.vector.memset(flags_sb[:], 0.0)

            # const bias tiles for activations
            cb_lnalpha = pp.tile([128, 1], F32, tag="cb_lnalpha")
            cb_fac = pp.tile([128, 1], F32, tag="cb_fac")
            cb_tiny = pp.tile([128, 1], F32, tag="cb_tiny")
            nc.vector.memset(cb_lnalpha[:], LN_ALPHA)
            nc.vector.memset(cb_fac[:], float(np.log(0.9) + 0.1 * np.log(256.0)))
            nc.vector.memset(cb_tiny[:], 1e-30)

            # persistent state tiles
            h_a = pp.tile([128, 2 * BC], F32, tag="h_a")
            h_b = pp.tile([128, 2 * BC], F32, tag="h_b")
            y_t = pp.tile([128, 2 * BC], F32, tag="y")
            ysave = pp.tile([128, 2 * BC], F32, tag="ysave")
            k1raw = pp.tile([128, 2 * BC], F32, tag="k1raw")
            # per-sample rows (persistent across a whole interval)
            tr = pp.tile([1, BC], F32, tag="tr")
            dtr = pp.tile([1, BC], F32, tag="dtr")

            nc.vector.memset(h_a[:], 0.0)

            # ---------------- GRU: 64 steps ----------------
            hc, hn_ = h_a, h_b
            for t in range(N_T):
                Rp = psA.tile([128, 2 * BC], F32, tag="psA")
                Zp = psA.tile([128, 2 * BC], F32, tag="psA")
                HNp = psA.tile([128, 2 * BC], F32, tag="psA")
                INp = psA.tile([128, 2 * BC], F32, tag="psA")
                mv = xm_sb[:, t * BC:(t + 1) * BC]
                # gate g (0=r,1=z,2=n), jc in {0,1}: Wih.T cols g*256+jc*128
                for g, (gi_ps, gh_ps) in enumerate([(Rp, Rp), (Zp, Zp), (INp, HNp)]):
                    for jc in range(2):
                        col = g * 256 + jc * 128
                        dst_i = gi_ps[:, jc * BC:(jc + 1) * BC]
                        dst_h = gh_ps[:, jc * BC:(jc + 1) * BC]
                        if g < 2:
                            # accumulate gi + gh into one psum slice
                            nc.tensor.matmul(dst_i, wih_sb[:, col:col + 128], mv,
                                             start=True, stop=False)
                            for ic in range(2):
                                nc.tensor.matmul(
                                    dst_h, whh_sb[:, ic * GW + col:ic * GW + col + 128],
                                    hc[:, ic * BC:(ic + 1) * BC],
                                    start=False, stop=(ic == 1))
                        else:
                            nc.tensor.matmul(dst_i, wih_sb[:, col:col + 128], mv,
                                             start=True, stop=True)
                            for ic in range(2):
                                nc.tensor.matmul(
                                    dst_h, whh_sb[:, ic * GW + col:ic * GW + col + 128],
                                    hc[:, ic * BC:(ic + 1) * BC],
                                    start=(ic == 0), stop=(ic == 1))
                r_t = wp.tile([128, 2 * BC], F32, tag="gr", bufs=2)
                z_t = wp.tile([128, 2 * BC], F32, tag="gz", bufs=2)
                n_t = wp.tile([128, 2 * BC], F32, tag="gn", bufs=2)
                tt1 = wp.tile([128, 2 * BC], F32, tag="gt1", bufs=2)
                tt2 = wp.tile([128, 2 * BC], F32, tag="gt2", bufs=2)
                d1 = wp.tile([128, 2 * BC], F32, tag="gd1", bufs=2)
                d2 = wp.tile([128, 2 * BC], F32, tag="gd2", bufs=2)
                nc.scalar.activation(r_t[:], Rp[:], AF.Sigmoid)
                nc.scalar.activation(z_t[:], Zp[:], AF.Sigmoid)
                nc.vector.tensor_tensor(out=tt1[:], in0=r_t[:], in1=HNp[:], op=ALU.mult)
                nc.vector.tensor_tensor(out=tt2[:], in0=tt1[:], in1=INp[:], op=ALU.add)
                nc.scalar.activation(n_t[:], tt2[:], AF.Tanh)
                nc.vector.tensor_tensor(out=d1[:], in0=hc[:], in1=n_t[:], op=ALU.subtract)
                nc.gpsimd.tensor_tensor(out=d2[:], in0=z_t[:], in1=d1[:], op=ALU.mult)
                nc.vector.tensor_tensor(out=hn_[:], in0=n_t[:], in1=d2[:], op=ALU.add)
                hc, hn_ = hn_, hc

            # ---------------- encoder head ----------------
            E1p = psA.tile([128, 2 * BC], F32, tag="psA")
            for jc in range(2):
                for ic in range(2):
                    nc.tensor.matmul(E1p[:, jc * BC:(jc + 1) * BC],
                                     e1_sb[:, (ic * 2 + jc) * 128:(ic * 2 + jc) * 128 + 128],
                                     hc[:, ic * BC:(ic + 1) * BC],
                                     start=(ic == 0), stop=(ic == 1))
            r1 = wp.tile([128, 2 * BC], F32, tag="r1", bufs=2)
            nc.scalar.activation(r1[:], E1p[:], AF.Relu)
            Mp = psA.tile([128, 2 * BC], F32, tag="psA")
            Sp = psA.tile([128, 2 * BC], F32, tag="psA")
            for jc in range(4):  # 0,1 -> mean; 2,3 -> std
                ps = Mp if jc < 2 else Sp
                jj = jc % 2
                for ic in range(2):
                    nc.tensor.matmul(ps[:, jj * BC:(jj + 1) * BC],
                                     e2_sb[:, (ic * 4 + jc) * 128:(ic * 4 + jc) * 128 + 128],
                                     r1[:, ic * BC:(ic + 1) * BC],
                                     start=(ic == 0), stop=(ic == 1))
            teps = wp.tile([128, 2 * BC], F32, tag="teps", bufs=2)
            nc.vector.tensor_tensor(out=teps[:], in0=eps_sb[:], in1=Sp[:], op=ALU.mult)
            nc.vector.tensor_tensor(out=y_t[:], in0=teps[:], in1=Mp[:], op=ALU.add)

            # ---------------- ODE machinery ----------------
            def bcast_mm(row_ap):
                """[1, BC] row -> PSUM [128, 2*BC] (same value for both chunks)."""
                ps = psB.tile([128, 2 * BC], F32, tag="psB")
                rhs = row_ap.rearrange("p (o b) -> p o b", o=1).broadcast_to([1, 2, BC])
                nc.tensor.matmul(ps[:], onesr[:], rhs, start=True, stop=True)
                return ps

            def feval(inp_tile):
                """4-layer SELU MLP (feature-major). Returns final-layer PSUM."""
                cur = inp_tile
                for l in range(4):
                    zp = psA.tile([128, 2 * BC], F32, tag="psA")
                    for jc in range(2):
                        for ic in range(2):
                            nc.tensor.matmul(
                                zp[:, jc * BC:(jc + 1) * BC],
                                ow_sb[l][:, (ic * 2 + jc) * 128:(ic * 2 + jc) * 128 + 128],
                                cur[:, ic * BC:(ic + 1) * BC],
                                start=(ic == 0), stop=(ic == 1))
                    if l < 3:
                        e2t = wp.tile([128, 2 * BC], F32, tag="selu_e", bufs=3)
                        ut = wp.tile([128, 2 * BC], F32, tag="selu_u", bufs=3)
                        ht = wp.tile([128, 2 * BC], F32, tag="selu_h", bufs=3)
                        nc.scalar.activation(e2t[:], zp[:], AF.Exp, bias=cb_lnalpha[:, :])
                        nc.gpsimd.tensor_scalar(out=ut[:], in0=e2t[:], scalar1=SELU_A,
                                                scalar2=SELU_A, op0=ALU.min,
                                                op1=ALU.subtract)
                        nc.vector.scalar_tensor_tensor(out=ht[:], in0=zp[:], scalar=0.0,
                                                       in1=ut[:], op0=ALU.max, op1=ALU.add)
                        cur = ht
                return zp

            STT = nc.vector.scalar_tensor_tensor
            TT = nc.vector.tensor_tensor
            TS = nc.vector.tensor_scalar

            def STTP(out, in0, scalar, in1, op0, op1):
                # Pool-legal decomposition of (in0 * scalar) + in1
                assert op0 == ALU.mult and op1 == ALU.add
                tmp = wp.tile([128, 2 * BC], F32, tag="pool_tmp", name="pool_tmp",
                              bufs=4)
                nc.gpsimd.tensor_scalar_mul(tmp[:], in0, scalar)
                nc.gpsimd.tensor_tensor(out=out, in0=tmp[:], in1=in1, op=ALU.add)

            def row(tag):
                return wp.tile([1, BC], F32, tag=tag, name=tag, bufs=2)

            def big(tag):
                return wp.tile([128, 2 * BC], F32, tag=tag, name=tag)

            for j in range(NI if ODE_ON else 0):
                t0row = pp.tile([1, BC], F32, tag=f"t0row{j}")
                t1row = pp.tile([1, BC], F32, tag=f"t1row{j}")
                dsrow = pp.tile([1, BC], F32, tag=f"dsrow{j}")
                nc.sync.dma_start(out=t0row[:], in_=t0r_d[j:j + 1, :])
                nc.sync.dma_start(out=t1row[:], in_=t1r_d[j:j + 1, :])
                nc.sync.dma_start(out=dsrow[:], in_=dsr_d[j:j + 1, :])
                t0j, t1j = t0row[:, :], t1row[:, :]
                # y_save = y;  y += dose (broadcast)
                nc.scalar.copy(ysave[:], y_t[:])
                DSp = bcast_mm(dsrow[:, :])
                nc.vector.tensor_tensor(out=y_t[:], in0=y_t[:], in1=DSp[:], op=ALU.add)
                # t = t0; dt = max(t1-t0,1e-6)*0.1
                nc.vector.tensor_copy(tr[:], t0j)
                du = row("du")
                nc.vector.tensor_tensor(out=du[:], in0=t1j, in1=t0j, op=ALU.subtract)
                TS(out=dtr[:], in0=du[:], scalar1=1e-6, scalar2=0.1,
                   op0=ALU.max, op1=ALU.mult)
                # k1 = f(y)
                zlast = feval(y_t)
                nc.scalar.copy(k1raw[:], zlast[:])

                for s in range(S):
                    # rows: active, dtc
                    active = row("active")
                    nc.vector.tensor_tensor(out=active[:], in0=tr[:], in1=t1j, op=ALU.is_lt)
                    u1 = row("u1")
                    nc.vector.tensor_tensor(out=u1[:], in0=t1j, in1=tr[:], op=ALU.subtract)
                    u2 = row("u2")
                    TS(out=u2[:], in0=u1[:], scalar1=0.0, scalar2=None, op0=ALU.max)
                    dtc = row("dtc")
                    nc.vector.tensor_tensor(out=dtc[:], in0=dtr[:], in1=u2[:], op=ALU.min)
                    Dp = bcast_mm(dtc[:, :])
                    Dsb = big("Dsb")
                    nc.scalar.copy(Dsb[:], Dp[:])

                    k1p = big("k1p")
                    nc.vector.tensor_tensor(out=k1p[:], in0=k1raw[:], in1=Dsb[:], op=ALU.mult)

                    # incremental accumulators
                    s2 = big("s2")
                    STT(out=s2[:], in0=k1p[:], scalar=A21, in1=y_t[:], op0=ALU.mult, op1=ALU.add)
                    s3a = big("s3a")
                    STTP(out=s3a[:], in0=k1p[:], scalar=A31, in1=y_t[:], op0=ALU.mult, op1=ALU.add)
                    s4a = big("s4a")
                    STTP(out=s4a[:], in0=k1p[:], scalar=A41, in1=y_t[:], op0=ALU.mult, op1=ALU.add)
                    s5a = big("s5a")
                    STTP(out=s5a[:], in0=k1p[:], scalar=A51, in1=y_t[:], op0=ALU.mult, op1=ALU.add)
                    s6a = big("s6a")
                    STTP(out=s6a[:], in0=k1p[:], scalar=A61, in1=y_t[:], op0=ALU.mult, op1=ALU.add)
                    y5a = big("y5a")
                    STTP(out=y5a[:], in0=k1p[:], scalar=BCO[0], in1=y_t[:], op0=ALU.mult, op1=ALU.add)
                    erra = big("erra")
                    TS(out=erra[:], in0=k1p[:], scalar1=EC[0], scalar2=None, op0=ALU.mult)

                    # k2
                    z2 = feval(s2)
                    k2p = big("k2p")
                    nc.vector.tensor_tensor(out=k2p[:], in0=z2[:], in1=Dsb[:], op=ALU.mult)
                    s3 = big("s3")
                    STT(out=s3[:], in0=k2p[:], scalar=A32, in1=s3a[:], op0=ALU.mult, op1=ALU.add)
                    s4b = big("s4b")
                    STTP(out=s4b[:], in0=k2p[:], scalar=A42, in1=s4a[:], op0=ALU.mult, op1=ALU.add)
                    s5b = big("s5b")
                    STTP(out=s5b[:], in0=k2p[:], scalar=A52, in1=s5a[:], op0=ALU.mult, op1=ALU.add)
                    s6b = big("s6b")
                    STTP(out=s6b[:], in0=k2p[:], scalar=A62, in1=s6a[:], op0=ALU.mult, op1=ALU.add)

                    # k3
                    z3 = feval(s3)
                    k3p = big("k3p")
                    nc.vector.tensor_tensor(out=k3p[:], in0=z3[:], in1=Dsb[:], op=ALU.mult)
                    s4 = big("s4")
                    STT(out=s4[:], in0=k3p[:], scalar=A43, in1=s4b[:], op0=ALU.mult, op1=ALU.add)
                    s5c = big("s5c")
                    STTP(out=s5c[:], in0=k3p[:], scalar=A53, in1=s5b[:], op0=ALU.mult, op1=ALU.add)
                    s6c = big("s6c")
                    STTP(out=s6c[:], in0=k3p[:], scalar=A63, in1=s6b[:], op0=ALU.mult, op1=ALU.add)
                    y5b = big("y5b")
                    STTP(out=y5b[:], in0=k3p[:], scalar=BCO[2], in1=y5a[:], op0=ALU.mult, op1=ALU.add)
                    errb = big("errb")
                    STTP(out=errb[:], in0=k3p[:], scalar=EC[2], in1=erra[:], op0=ALU.mult, op1=ALU.add)

                    # k4
                    z4 = feval(s4)
                    k4p = big("k4p")
                    nc.vector.tensor_tensor(out=k4p[:], in0=z4[:], in1=Dsb[:], op=ALU.mult)
                    s5 = big("s5")
                    STT(out=s5[:], in0=k4p[:], scalar=A54, in1=s5c[:], op0=ALU.mult, op1=ALU.add)
                    s6d = big("s6d")
                    STTP(out=s6d[:], in0=k4p[:], scalar=A64, in1=s6c[:], op0=ALU.mult, op1=ALU.add)
                    y5c = big("y5c")
                    STTP(out=y5c[:], in0=k4p[:], scalar=BCO[3], in1=y5b[:], op0=ALU.mult, op1=ALU.add)
                    errc = big("errc")
                    STTP(out=errc[:], in0=k4p[:], scalar=EC[3], in1=errb[:], op0=ALU.mult, op1=ALU.add)

                    # k5
                    z5 = feval(s5)
                    k5p = big("k5p")
                    nc.vector.tensor_tensor(out=k5p[:], in0=z5[:], in1=Dsb[:], op=ALU.mult)
                    s6 = big("s6")
                    STT(out=s6[:], in0=k5p[:], scalar=A65, in1=s6d[:], op0=ALU.mult, op1=ALU.add)
                    y5d = big("y5d")
                    STTP(out=y5d[:], in0=k5p[:], scalar=BCO[4], in1=y5c[:], op0=ALU.mult, op1=ALU.add)
                    errd = big("errd")
                    STTP(out=errd[:], in0=k5p[:], scalar=EC[4], in1=errc[:], op0=ALU.mult, op1=ALU.add)

                    # k6
                    z6 = feval(s6)
                    k6p = big("k6p")
                    nc.vector.tensor_tensor(out=k6p[:], in0=z6[:], in1=Dsb[:], op=ALU.mult)
                    y5 = big("y5")
                    STT(out=y5[:], in0=k6p[:], scalar=BCO[5], in1=y5d[:], op0=ALU.mult, op1=ALU.add)
                    erre = big("erre")
                    STTP(out=erre[:], in0=k6p[:], scalar=EC[5], in1=errd[:], op0=ALU.mult, op1=ALU.add)

                    # sc terms overlap with k7 eval
                    m1 = big("m1")
                    TT(out=m1[:], in0=y_t[:], in1=y5[:], op=ALU.max)
                    m2 = big("m2")
                    TT(out=m2[:], in0=y_t[:], in1=y5[:], op=ALU.min)
                    wmx = big("wmx")
                    STT(out=wmx[:], in0=m2[:], scalar=-1.0, in1=m1[:], op0=ALU.mult, op1=ALU.max)
                    scv = big("scv")
                    TS(out=scv[:], in0=wmx[:], scalar1=RTOL, scalar2=ATOL,
                       op0=ALU.mult, op1=ALU.add)
                    rsc = big("rsc")
                    nc.vector.reciprocal(rsc[:], scv[:])

                    # k7 = f(y5)
                    z7 = feval(y5)
                    k7r = big("k7r")
                    nc.scalar.copy(k7r[:], z7[:])
                    k7p = big("k7p")
                    nc.vector.tensor_tensor(out=k7p[:], in0=z7[:], in1=Dsb[:], op=ALU.mult)
                    errf = big("errf")
                    STT(out=errf[:], in0=k7p[:], scalar=EC[6], in1=erre[:], op0=ALU.mult, op1=ALU.add)
                    vv = big("vv")
                    nc.vector.tensor_tensor(out=vv[:], in0=errf[:], in1=rsc[:], op=ALU.mult)
                    vsq = big("vsq")
                    nc.scalar.square(vsq[:], vv[:])
                    qp = psQ.tile([1, 2 * BC], F32, tag="psQ")
                    nc.tensor.matmul(qp[:], onesc[:], vsq[:], start=True, stop=True)
                    qsb = wp.tile([1, 2 * BC], F32, tag="qsb", name="qsb", bufs=2)
                    nc.scalar.copy(qsb[:], qp[:])
                    qs = row("qs")
                    nc.vector.tensor_tensor(out=qs[:], in0=qsb[:, 0:BC], in1=qsb[:, BC:2 * BC],
                                            op=ALU.add)
                    # acc = (sum_sq <= 256) & active   (en = sqrt(q/256) <= 1)
                    accr = row("accr")
                    TS(out=accr[:], in0=qs[:], scalar1=256.0, scalar2=None, op0=ALU.is_le)
                    nc.vector.tensor_tensor(out=accr[:], in0=accr[:], in1=active[:], op=ALU.mult)
                    # fac = clip(0.9*(q/256)^-0.1, 0.2, 10) via ln/exp
                    lq = row("lq")
                    nc.scalar.activation(lq[:], qs[:], AF.Ln, bias=cb_tiny[:1, :])
                    f0 = row("f0")
                    nc.scalar.activation(f0[:], lq[:], AF.Exp, scale=-0.1,
                                         bias=cb_fac[:1, :])
                    fac = row("fac")
                    TS(out=fac[:], in0=f0[:], scalar1=0.2, scalar2=10.0,
                       op0=ALU.max, op1=ALU.min)

                    # accept broadcast + state updates
                    ACCp = bcast_mm(accr[:, :])
                    accu = wp.tile([128, 2 * BC], U8, tag="accu8", bufs=2)
                    nc.vector.tensor_copy(accu[:], ACCp[:])
                    nc.vector.copy_predicated(y_t[:], accu[:], y5[:])
                    nc.vector.copy_predicated(k1raw[:], accu[:], k7r[:])
                    # t += acc*dtc ; dt = active? max(dtc,1e-8)*fac : dt
                    q1 = row("q1")
                    nc.vector.tensor_tensor(out=q1[:], in0=dtc[:], in1=accr[:], op=ALU.mult)
                    nc.vector.tensor_tensor(out=tr[:], in0=tr[:], in1=q1[:], op=ALU.add)
                    md = row("md")
                    TS(out=md[:], in0=dtc[:], scalar1=1e-8, scalar2=None, op0=ALU.max)
                    nd = row("nd")
                    nc.vector.tensor_tensor(out=nd[:], in0=md[:], in1=fac[:], op=ALU.mult)
                    dd = row("dd")
                    nc.vector.tensor_tensor(out=dd[:], in0=nd[:], in1=dtr[:], op=ALU.subtract)
                    da = row("da")
                    nc.vector.tensor_tensor(out=da[:], in0=dd[:], in1=active[:], op=ALU.mult)
                    nc.vector.tensor_tensor(out=dtr[:], in0=dtr[:], in1=da[:], op=ALU.add)

                # flag_j = any(t < t1) after S steps
                af = row("af")
                nc.vector.tensor_tensor(out=af[:], in0=tr[:], in1=t1j, op=ALU.is_lt)
                nc.vector.tensor_reduce(out=flags_sb[:, j:j + 1], in_=af[:],
                                        axis=mybir.AxisListType.X, op=ALU.max)

                # restore y where interval was empty (t0 >= t1)
                a0i = row("a0i")
                nc.vector.tensor_tensor(out=a0i[:], in0=t0j, in1=t1j, op=ALU.is_ge)
                A0p = bcast_mm(a0i[:, :])
                a0u = wp.tile([128, 2 * BC], U8, tag="accu8", bufs=2)
                nc.vector.tensor_copy(a0u[:], A0p[:])
                nc.vector.copy_predicated(y_t[:], a0u[:], ysave[:])

            nc.sync.dma_start(out=yout_d[:, :], in_=y_t[:])
            nc.sync.dma_start(out=flags_d[:, :], in_=flags_sb[:])

    nc.compile()
    return nc


def _get_nc(S):
    if S not in _BUILD_CACHE:
        _BUILD_CACHE[S] = _build(S)
    return _BUILD_CACHE[S]


def _prep_inputs(inputs):
    """Host-side: fold SELU constants into weights, shard batch, relayout."""
    x = np.asarray(inputs["x"], np.float32)
    meta = np.asarray(inputs["meta"], np.float32)
    eps = np.asarray(inputs["eps"], np.float32)
    times_split = np.asarray(inputs["times_split"], np.float32)
    doses_split = np.asarray(inputs["doses_split"], np.float32)

    gw = {k: np.asarray(inputs[k], np.float32) for k in
          ("gru_Wih", "gru_Whh", "enc_W1", "enc_W2",
           "ode_W1", "ode_W2", "ode_W3", "ode_W4", "fc_W")}
    for k in ("gru_bih", "gru_bhh", "enc_b1", "enc_b2",
              "ode_b1", "ode_b2", "ode_b3", "ode_b4", "fc_b"):
        bv = np.asarray(inputs[k], np.float32)
        if np.any(bv != 0):
            raise NotImplementedError(f"nonzero bias {k} not supported by this kernel")

    # lambda folding: kernel's selu outputs selu(z)/lambda -> scale layers 2..4
    odeW = [gw["ode_W1"],
            SELU_L * gw["ode_W2"], SELU_L * gw["ode_W3"], SELU_L * gw["ode_W4"]]

    xm = np.concatenate(
        [x, np.broadcast_to(meta[:, None, :], (B, N_T, M_META))], axis=-1)  # [B,T,8]

    common = {
        "wihT": np.ascontiguousarray(gw["gru_Wih"].T),          # [8, 768]
        "whhT": _lhsT(gw["gru_Whh"]),                           # [128, 1536]
        "encW1T": _lhsT(gw["enc_W1"]),
        "encW2T": _lhsT(gw["enc_W2"]),
        "odeW1T": _lhsT(odeW[0]), "odeW2T": _lhsT(odeW[1]),
        "odeW3T": _lhsT(odeW[2]), "odeW4T": _lhsT(odeW[3]),
        "onesc": np.ones((128, 1), np.float32),
        "onesr": np.ones((1, BC), np.float32),
    }

    in_maps = []
    for c in range(NCORES):
        sl = slice(c * BC, (c + 1) * BC)
        xmc = xm[sl]                                            # [BC, T, 8]
        m = dict(common)
        m["xmT"] = np.ascontiguousarray(
            xmc.transpose(2, 1, 0).reshape(D_IN + M_META, N_T * BC))
        m["epsT"] = _chunked(np.ascontiguousarray(eps[sl].T))   # [128, 256]
        m["t0r"] = np.ascontiguousarray(times_split[:, sl, 0])
        m["t1r"] = np.ascontiguousarray(times_split[:, sl, 1])
        m["dsr"] = np.ascontiguousarray(doses_split[:, sl])
        in_maps.append(m)
    return in_maps, meta, gw["fc_W"], np.asarray(inputs["fc_b"], np.float32)


def _gather(results):
    """Per-core yout [128, 2*BC] -> full y [B, H]."""
    ys = []
    for r in results:
        t = r["yout"].reshape(128, 2, BC)
        ys.append(np.ascontiguousarray(t.transpose(2, 1, 0).reshape(BC, H)))
    return np.concatenate(ys, axis=0)


def kernel(**inputs):
    from concourse.bass_utils import run_bass_kernel_spmd

    in_maps, meta, fc_W, fc_b = _prep_inputs(inputs)
    core_ids = list(range(NCORES))

    res = run_bass_kernel_spmd(_get_nc(S_FAST), in_maps, core_ids)
    LAST_RESULTS.clear()
    LAST_RESULTS.append(res)
    flags = np.concatenate([r["flags"] for r in res.results])
    if np.any(flags > 0):
        res = run_bass_kernel_spmd(_get_nc(MAX_STEPS), in_maps, core_ids)
        LAST_RESULTS.append(res)

    y = _gather(res.results)
    latent = np.concatenate([y, meta], axis=1)
    return (latent @ fc_W.T + fc_b).astype(np.float32)
